# revision 1
# baseline (speedup 1.0000x reference)
"""Cross-attention kernel for Trainium2 (8 NeuronCores, batch-parallel).

Math per batch b (reference semantics):
  q = queries[b].reshape(C, N).T + q_pos        # [N, C]
  k = keys[b].reshape(C, N).T + k_pos
  v = values[b].reshape(C, N).T                 # [N, C]
  out = softmax(q @ k.T / 16) @ v, returned as [C, N] (c-major)

Device layout (per core = one batch):
  All matmuls in f32r (TF32 mode, 1 PE cycle/row).  S is computed transposed
  (S^T[k, q]) so that exp(S^T) tiles are directly the rhs of the O matmul
  (O^T = V^T A^T) and the softmax denominator comes from a ones-column
  matmul -- no on-chip transposes anywhere.
"""

import numpy as np

import concourse.bass as bass
import concourse.tile as tile
import concourse.mybir as mybir
from concourse import bacc
from concourse.bass_utils import run_bass_kernel_spmd

P = 128          # partitions
C = 256          # qk/v channel dim
N = 4096         # sequence (64*64)
B = 8            # batch == n_cores
QW = 512         # query block width (max fp32-class matmul free dim)
NQB = N // QW    # 8 query blocks
NKO = N // P     # 32 key chunks
KPB = QW // P    # key chunks per K block tile
SCALE = 1.0 / 16.0  # 1/sqrt(C)

F32 = mybir.dt.float32
F32R = mybir.dt.float32r
AF = mybir.ActivationFunctionType

_NC_CACHE = None


def tf32_round(x: np.ndarray) -> np.ndarray:
    u = x.view(np.uint32)
    u = (u + np.uint32(0x1000)) & np.uint32(0xFFFFE000)
    return u.view(np.float32)


def build_nc(atp_bufs=6, raw_bufs=3, ps_s_bufs=4, ps_o_bufs=1, lag=3):
    nc = bacc.Bacc(None, target_bir_lowering=False)
    qt = nc.dram_tensor("qt", [C, N], F32, kind="ExternalInput")
    kt = nc.dram_tensor("kt", [C, N], F32, kind="ExternalInput")
    v = nc.dram_tensor("v", [N, C], F32R, kind="ExternalInput")
    qp = nc.dram_tensor("qp", [C, N], F32, kind="ExternalInput")
    kp = nc.dram_tensor("kp", [C, N], F32, kind="ExternalInput")
    o = nc.dram_tensor("o", [C, N], F32, kind="ExternalOutput")

    qt3 = qt.rearrange("(co p) n -> p co n", p=P)
    kt3 = kt.rearrange("(co p) n -> p co n", p=P)
    qp3 = qp.rearrange("(co p) n -> p co n", p=P)
    kp3 = kp.rearrange("(co p) n -> p co n", p=P)
    v3 = v.rearrange("(ko p) c -> p ko c", p=P)

    with tile.TileContext(nc) as tc:
        with (
            tc.tile_pool(name="consts", bufs=1) as consts,
            tc.tile_pool(name="qk", bufs=NQB) as qk,
            tc.tile_pool(name="vp", bufs=NKO) as vp,
            tc.tile_pool(name="raw", bufs=raw_bufs) as raw,
            tc.tile_pool(name="atp", bufs=atp_bufs) as atp,
            tc.tile_pool(name="small", bufs=2) as small,
            tc.tile_pool(name="outp", bufs=2) as outp,
            tc.tile_pool(name="ps_s", bufs=ps_s_bufs, space="PSUM") as ps_s,
            tc.tile_pool(name="ps_o", bufs=ps_o_bufs, space="PSUM") as ps_o,
            tc.tile_pool(name="ps_r", bufs=1, space="PSUM") as ps_r,
            tc.tile_pool(name="ps_b", bufs=1, space="PSUM") as ps_b,
        ):
            ones_f = consts.tile([P, 2], F32, tag="ones_f")
            nc.vector.memset(ones_f, 1.0)
            ones_c = consts.tile([P, 2], F32R, tag="ones_c")
            nc.vector.tensor_copy(ones_c, ones_f)
            ones_rf = consts.tile([1, P], F32, tag="ones_rf")
            nc.vector.memset(ones_rf, 1.0)
            ones_r = consts.tile([1, P], F32R, tag="ones_r")
            nc.vector.tensor_copy(ones_r, ones_rf)

            # K blocks (pos-added, f32r) and V chunks, emitted in deadline
            # order: block 0's dependencies first (K0, V0..3), then K(jb)
            # interleaved with the V chunks needed just before it.
            def load_kblk(j):
                sl = slice(j * QW, (j + 1) * QW)
                kraw = raw.tile([P, 2, QW], F32, tag="kraw")
                kpos = raw.tile([P, 2, QW], F32, tag="kpos")
                nc.sync.dma_start(kraw, kt3[:, :, sl])
                nc.sync.dma_start(kpos, kp3[:, :, sl])
                kb = qk.tile([P, 2, QW], F32R, tag="kblk")
                if j == 0:
                    for co in range(2):
                        nc.vector.tensor_add(kb[:, co, :], kraw[:, co, :],
                                             kpos[:, co, :])
                else:
                    nc.vector.tensor_add(kb, kraw, kpos)
                return kb

            def load_vchunk(ko):
                vc = vp.tile([P, C], F32R, tag="v")
                nc.sync.dma_start(vc, v3[:, ko, :])
                return vc

            kblks = {}
            vcs = {}
            kblks[0] = load_kblk(0)

            def emit_epilogue(j, po0, po1, pr):
                sl = slice(j * QW, (j + 1) * QW)
                inv = small.tile([1, QW], F32R, tag="inv")
                with nc.allow_low_precision(
                    reason="TF32 rounding of softmax reciprocal"
                ):
                    nc.vector.reciprocal(inv, pr[0:1, :])
                pb = ps_b.tile([P, QW], F32, tag="b")
                nc.tensor.matmul(pb, ones_r, inv, start=True, stop=True)
                bs = small.tile([P, QW], F32, tag="bs")
                nc.vector.tensor_copy(bs, pb)
                oo0 = outp.tile([P, QW], F32, tag="oo0")
                nc.vector.tensor_mul(oo0, po0, bs)
                nc.sync.dma_start(o[0:P, sl], oo0)
                oo1 = outp.tile([P, QW], F32, tag="oo1")
                nc.vector.tensor_mul(oo1, po1, bs)
                nc.sync.dma_start(o[P:C, sl], oo1)

            pending = None
            for j in range(NQB):
                sl = slice(j * QW, (j + 1) * QW)
                qraw = raw.tile([P, 2, QW], F32, tag="qraw")
                qpos = raw.tile([P, 2, QW], F32, tag="qpos")
                nc.sync.dma_start(qraw, qt3[:, :, sl])
                nc.sync.dma_start(qpos, qp3[:, :, sl])
                qb = qk.tile([P, 2, QW], F32R, tag="qblk")
                if j == 0:
                    for co in range(2):
                        nc.vector.tensor_add(qb[:, co, :], qraw[:, co, :],
                                             qpos[:, co, :])
                else:
                    nc.vector.tensor_add(qb, qraw, qpos)

                if j == 0:
                    # deadline-ordered remaining loads: V(4jb..) then K(jb+1)
                    for jb in range(NQB):
                        for ko in range(4 * jb, 4 * jb + 4):
                            vcs[ko] = load_vchunk(ko)
                        if jb + 1 < NQB:
                            kblks[jb + 1] = load_kblk(jb + 1)

                po0 = ps_o.tile([P, QW], F32, tag="o0")
                po1 = ps_o.tile([P, QW], F32, tag="o1")
                pr = ps_r.tile([2, QW], F32, tag="r")

                a_q = {}

                for ko in range(NKO):
                    pss = ps_s.tile([P, QW], F32, tag="s")
                    jb, koff = divmod(ko, KPB)
                    for co in range(2):
                        nc.tensor.matmul(
                            pss,
                            kblks[jb][:, co, koff * P : (koff + 1) * P],
                            qb[:, co, :],
                            start=(co == 0),
                            stop=(co == 1),
                        )
                    a = atp.tile([P, QW], F32R, tag="a")
                    nc.scalar.activation(a, pss, AF.Exp, scale=SCALE)
                    a_q[ko] = a

                    if ko >= lag:
                        pko = ko - lag
                        av = a_q[pko]
                        nc.tensor.matmul(po0, vcs[pko][:, 0:P], av,
                                         start=(pko == 0), stop=False)
                        nc.tensor.matmul(po1, vcs[pko][:, P:C], av,
                                         start=(pko == 0), stop=False)
                        nc.tensor.matmul(pr, ones_c, av,
                                         start=(pko == 0), stop=False)
                        del a_q[pko]

                    if ko == 2 and pending is not None:
                        emit_epilogue(*pending)
                        pending = None

                # drain remaining lagged chunks; last closes the groups
                for pko in range(NKO - lag, NKO):
                    av = a_q[pko]
                    last = pko == NKO - 1
                    nc.tensor.matmul(po0, vcs[pko][:, 0:P], av,
                                     start=False, stop=last)
                    nc.tensor.matmul(po1, vcs[pko][:, P:C], av,
                                     start=False, stop=last)
                    nc.tensor.matmul(pr, ones_c, av, start=False, stop=last)
                    del a_q[pko]
                pending = (j, po0, po1, pr)

            emit_epilogue(*pending)

    nc.compile()
    return nc


def _get_nc():
    global _NC_CACHE
    if _NC_CACHE is None:
        _NC_CACHE = build_nc()
    return _NC_CACHE


def make_in_maps(queries, keys, values, q_pos_embedding, k_pos_embedding):
    queries = np.asarray(queries, dtype=np.float32)
    keys = np.asarray(keys, dtype=np.float32)
    values = np.asarray(values, dtype=np.float32)
    qpT = np.ascontiguousarray(
        np.asarray(q_pos_embedding, dtype=np.float32).reshape(N, C).T
    )
    kpT = np.ascontiguousarray(
        np.asarray(k_pos_embedding, dtype=np.float32).reshape(N, C).T
    )
    in_maps = []
    for b in range(B):
        vT = tf32_round(
            np.ascontiguousarray(values[b].reshape(C, N).T)
        )
        in_maps.append({
            "qt": np.ascontiguousarray(queries[b].reshape(C, N)),
            "kt": np.ascontiguousarray(keys[b].reshape(C, N)),
            "v": vT,
            "qp": qpT,
            "kp": kpT,
        })
    return in_maps


def kernel(queries, keys, values, q_pos_embedding, k_pos_embedding):
    nc = _get_nc()
    in_maps = make_in_maps(queries, keys, values, q_pos_embedding,
                           k_pos_embedding)
    res = run_bass_kernel_spmd(nc, in_maps, core_ids=list(range(B)))
    out = np.stack([r["o"].reshape(C, 64, 64) for r in res.results])
    return out.astype(np.float32)


def build_nc_trivial():
    """Same I/O signature, minimal work: used by test.py to subtract the
    per-call transfer/dispatch overhead from wall-clock timing."""
    nc = bacc.Bacc(None, target_bir_lowering=False)
    qt = nc.dram_tensor("qt", [C, N], F32, kind="ExternalInput")
    kt = nc.dram_tensor("kt", [C, N], F32, kind="ExternalInput")
    v = nc.dram_tensor("v", [N, C], F32R, kind="ExternalInput")
    qp = nc.dram_tensor("qp", [C, N], F32, kind="ExternalInput")
    kp = nc.dram_tensor("kp", [C, N], F32, kind="ExternalInput")
    o = nc.dram_tensor("o", [C, N], F32, kind="ExternalOutput")
    with tile.TileContext(nc) as tc:
        with tc.tile_pool(name="sb", bufs=2) as sb:
            t = sb.tile([P, 2, N], F32, tag="t")
            nc.sync.dma_start(t, qt.rearrange("(co p) n -> p co n", p=P))
            nc.sync.dma_start(o.rearrange("(co p) n -> p co n", p=P), t)
    nc.compile()
    return nc



# revision 2
# speedup vs baseline: 1.2774x; 1.2774x over previous
"""Cross-attention kernel for Trainium2 (8 NeuronCores, batch-parallel).

Math per batch b (reference semantics):
  q = queries[b].reshape(C, N).T + q_pos        # [N, C]
  k = keys[b].reshape(C, N).T + k_pos
  v = values[b].reshape(C, N).T                 # [N, C]
  out = softmax(q @ k.T / 16) @ v, returned as [C, N] (c-major)

Device layout (per core = one batch):
  S is computed transposed (S^T[k, q]) so that exp(S^T) tiles are directly
  the rhs of the O matmul (O^T = V^T A^T) -- no on-chip transposes.

  S matmuls run as error-compensated fp8 (e4m3) in DoubleRow perf mode:
  q = q_hi + q_lo, k = k_hi + k_lo (hi = fp8 round, lo = fp8 residual) and
  S ~= Qh Kh + Ql Kh + Qh Kl (the dropped Ql Kl term is O(2^-8) relative).
  Each DoubleRow matmul contracts all 256 channels in one instruction.

  O matmuls use bf16 A (exp output) against bf16 V.  The softmax
  denominator is accumulated on the vector engine in fp16 (2x DVE mode)
  instead of burning PE streams on ones-matmuls per key chunk; a single
  ones-matmul per query block folds the 128 partitions.
"""

import numpy as np
import ml_dtypes

import concourse.bass as bass
import concourse.tile as tile
import concourse.mybir as mybir
from concourse import bacc
from concourse.bass_utils import run_bass_kernel_spmd

P = 128          # partitions
C = 256          # qk/v channel dim
N = 4096         # sequence (64*64)
B = 8            # batch == n_cores
QW = 512         # query block width (max fp32-class matmul free dim)
NQB = N // QW    # 8 query blocks
NKO = N // P     # 32 key chunks
KPB = QW // P    # key chunks per K block tile
SCALE = 1.0 / 16.0  # 1/sqrt(C)

F32 = mybir.dt.float32
F32R = mybir.dt.float32r
BF16 = mybir.dt.bfloat16
FP16 = mybir.dt.float16
FP8 = mybir.dt.float8e4
AF = mybir.ActivationFunctionType
DR = mybir.MatmulPerfMode.DoubleRow

_NC_CACHE = None


def build_nc(atp_bufs=6, raw_bufs=3, ps_s_bufs=3, ps_o_bufs=1, lag=3):
    nc = bacc.Bacc(None, target_bir_lowering=False)
    qt = nc.dram_tensor("qt", [C, N], BF16, kind="ExternalInput")
    kt = nc.dram_tensor("kt", [C, N], BF16, kind="ExternalInput")
    v = nc.dram_tensor("v", [N, C], BF16, kind="ExternalInput")
    qp = nc.dram_tensor("qp", [C, N], BF16, kind="ExternalInput")
    kp = nc.dram_tensor("kp", [C, N], BF16, kind="ExternalInput")
    o = nc.dram_tensor("o", [C, N], F32, kind="ExternalOutput")

    qt3 = qt.rearrange("(co p) n -> p co n", p=P)
    kt3 = kt.rearrange("(co p) n -> p co n", p=P)
    qp3 = qp.rearrange("(co p) n -> p co n", p=P)
    kp3 = kp.rearrange("(co p) n -> p co n", p=P)
    v3 = v.rearrange("(ko p) c -> p ko c", p=P)

    with tile.TileContext(nc) as tc:
        with (
            tc.tile_pool(name="consts", bufs=1) as consts,
            tc.tile_pool(name="ksplit", bufs=NQB) as ksplit,
            tc.tile_pool(name="qsplit", bufs=3) as qsplit,
            tc.tile_pool(name="sums", bufs=2) as sums,
            tc.tile_pool(name="vp", bufs=NKO) as vp,
            tc.tile_pool(name="raw", bufs=raw_bufs) as raw,
            tc.tile_pool(name="atp", bufs=atp_bufs) as atp,
            tc.tile_pool(name="accp", bufs=2) as accp,
            tc.tile_pool(name="small", bufs=2) as small,
            tc.tile_pool(name="outp", bufs=2) as outp,
            tc.tile_pool(name="ps_s", bufs=ps_s_bufs, space="PSUM") as ps_s,
            tc.tile_pool(name="ps_o", bufs=ps_o_bufs, space="PSUM") as ps_o,
            tc.tile_pool(name="ps_r", bufs=1, space="PSUM") as ps_r,
            tc.tile_pool(name="ps_b", bufs=1, space="PSUM") as ps_b,
        ):
            ones_f = consts.tile([P, 2], F32, tag="ones_f")
            nc.vector.memset(ones_f, 1.0)
            ones_h = consts.tile([P, 2], FP16, tag="ones_h")
            nc.vector.tensor_copy(ones_h, ones_f)
            ones_rf = consts.tile([1, P], F32, tag="ones_rf")
            nc.vector.memset(ones_rf, 1.0)
            ones_r = consts.tile([1, P], F32R, tag="ones_r")
            nc.vector.tensor_copy(ones_r, ones_rf)

            # K blocks (pos-added, fp8 hi/lo split) and V chunks, emitted in
            # deadline order: block 0's dependencies first (K0, V0..3), then
            # K(jb) interleaved with the V chunks needed just before it.
            def load_kblk(j):
                sl = slice(j * QW, (j + 1) * QW)
                kraw = raw.tile([P, 2, QW], BF16, tag="kraw")
                kpos = raw.tile([P, 2, QW], BF16, tag="kpos")
                nc.sync.dma_start(kraw, kt3[:, :, sl])
                nc.sync.dma_start(kpos, kp3[:, :, sl])
                ksum = sums.tile([P, 2, QW], BF16, tag="ksum")
                if j == 0:
                    for co in range(2):
                        nc.vector.tensor_add(ksum[:, co, :], kraw[:, co, :],
                                             kpos[:, co, :])
                else:
                    nc.vector.tensor_add(ksum, kraw, kpos)
                khi = ksplit.tile([P, 2, QW], FP8, tag="khi")
                nc.vector.tensor_copy(khi, ksum)
                klo = ksplit.tile([P, 2, QW], FP8, tag="klo")
                # klo = (khi * -1) + ksum
                nc.vector.scalar_tensor_tensor(
                    klo, khi, -1.0, ksum,
                    op0=mybir.AluOpType.mult, op1=mybir.AluOpType.add,
                )
                return khi, klo

            def load_vchunk(ko):
                vc = vp.tile([P, C], BF16, tag="v")
                nc.sync.dma_start(vc, v3[:, ko, :])
                return vc

            kblks = {}
            vcs = {}
            kblks[0] = load_kblk(0)

            def emit_epilogue(j, po0, po1, pr):
                sl = slice(j * QW, (j + 1) * QW)
                inv = small.tile([1, QW], F32R, tag="inv")
                with nc.allow_low_precision(
                    reason="TF32 rounding of softmax reciprocal"
                ):
                    nc.vector.reciprocal(inv, pr[0:1, :])
                pb = ps_b.tile([P, QW], F32, tag="b")
                nc.tensor.matmul(pb, ones_r, inv, start=True, stop=True)
                bs = small.tile([P, QW], F32, tag="bs")
                nc.vector.tensor_copy(bs, pb)
                oo0 = outp.tile([P, QW], F32, tag="oo0")
                nc.vector.tensor_mul(oo0, po0, bs)
                nc.sync.dma_start(o[0:P, sl], oo0)
                oo1 = outp.tile([P, QW], F32, tag="oo1")
                nc.vector.tensor_mul(oo1, po1, bs)
                nc.sync.dma_start(o[P:C, sl], oo1)

            pending = None
            for j in range(NQB):
                qraw = raw.tile([P, 2, QW], BF16, tag="qraw")
                qpos = raw.tile([P, 2, QW], BF16, tag="qpos")
                sl = slice(j * QW, (j + 1) * QW)
                nc.sync.dma_start(qraw, qt3[:, :, sl])
                nc.sync.dma_start(qpos, qp3[:, :, sl])
                qsum = sums.tile([P, 2, QW], BF16, tag="qsum")
                if j == 0:
                    for co in range(2):
                        nc.vector.tensor_add(qsum[:, co, :], qraw[:, co, :],
                                             qpos[:, co, :])
                else:
                    nc.vector.tensor_add(qsum, qraw, qpos)
                qhi = qsplit.tile([P, 2, QW], FP8, tag="qhi")
                nc.vector.tensor_copy(qhi, qsum)
                qlo = qsplit.tile([P, 2, QW], FP8, tag="qlo")
                nc.vector.scalar_tensor_tensor(
                    qlo, qhi, -1.0, qsum,
                    op0=mybir.AluOpType.mult, op1=mybir.AluOpType.add,
                )

                if j == 0:
                    # deadline-ordered remaining loads: V(4jb..) then K(jb+1)
                    for jb in range(NQB):
                        for ko in range(4 * jb, 4 * jb + 4):
                            vcs[ko] = load_vchunk(ko)
                        if jb + 1 < NQB:
                            kblks[jb + 1] = load_kblk(jb + 1)

                po0 = ps_o.tile([P, QW], F32, tag="o0")
                po1 = ps_o.tile([P, QW], F32, tag="o1")
                acc = accp.tile([P, QW], FP16, tag="acc")

                a_q = {}

                for ko in range(NKO):
                    pss = ps_s.tile([P, QW], F32, tag="s")
                    jb, koff = divmod(ko, KPB)
                    khi, klo = kblks[jb]
                    ksl = slice(koff * P, (koff + 1) * P)
                    nc.tensor.matmul(pss, khi[:, :, ksl], qhi,
                                     start=True, stop=False, perf_mode=DR)
                    nc.tensor.matmul(pss, khi[:, :, ksl], qlo,
                                     start=False, stop=False, perf_mode=DR)
                    nc.tensor.matmul(pss, klo[:, :, ksl], qhi,
                                     start=False, stop=True, perf_mode=DR)
                    a = atp.tile([P, QW], BF16, tag="a")
                    nc.scalar.activation(a, pss, AF.Exp, scale=SCALE)
                    a_q[ko] = a
                    # fp16 row-sum accumulator on DVE (2x mode: all 2-byte)
                    if ko == 0:
                        nc.vector.tensor_copy(acc, a)
                    else:
                        nc.vector.tensor_add(acc, acc, a)

                    if ko >= lag:
                        pko = ko - lag
                        av = a_q[pko]
                        nc.tensor.matmul(po0, vcs[pko][:, 0:P], av,
                                         start=(pko == 0), stop=False)
                        nc.tensor.matmul(po1, vcs[pko][:, P:C], av,
                                         start=(pko == 0),
                                         stop=(pko == NKO - 1))
                        del a_q[pko]

                    if ko == 2 and pending is not None:
                        emit_epilogue(*pending)
                        pending = None

                # drain remaining lagged chunks; last closes the groups
                for pko in range(NKO - lag, NKO):
                    av = a_q[pko]
                    last = pko == NKO - 1
                    nc.tensor.matmul(po0, vcs[pko][:, 0:P], av,
                                     start=False, stop=last)
                    nc.tensor.matmul(po1, vcs[pko][:, P:C], av,
                                     start=False, stop=last)
                    del a_q[pko]

                # fold the 128 partitions of acc into the denominator row
                pr = ps_r.tile([2, QW], F32, tag="r")
                nc.tensor.matmul(pr, ones_h, acc, start=True, stop=True)
                pending = (j, po0, po1, pr)

            emit_epilogue(*pending)

    nc.compile()
    return nc


def _get_nc():
    global _NC_CACHE
    if _NC_CACHE is None:
        _NC_CACHE = build_nc()
    return _NC_CACHE


def make_in_maps(queries, keys, values, q_pos_embedding, k_pos_embedding):
    bf16 = ml_dtypes.bfloat16
    queries = np.asarray(queries, dtype=np.float32)
    keys = np.asarray(keys, dtype=np.float32)
    values = np.asarray(values, dtype=np.float32)
    qpT = np.ascontiguousarray(
        np.asarray(q_pos_embedding, dtype=np.float32).reshape(N, C).T
    ).astype(bf16)
    kpT = np.ascontiguousarray(
        np.asarray(k_pos_embedding, dtype=np.float32).reshape(N, C).T
    ).astype(bf16)
    in_maps = []
    for b in range(B):
        vT = np.ascontiguousarray(values[b].reshape(C, N).T).astype(bf16)
        in_maps.append({
            "qt": np.ascontiguousarray(queries[b].reshape(C, N)).astype(bf16),
            "kt": np.ascontiguousarray(keys[b].reshape(C, N)).astype(bf16),
            "v": vT,
            "qp": qpT,
            "kp": kpT,
        })
    return in_maps


def kernel(queries, keys, values, q_pos_embedding, k_pos_embedding):
    nc = _get_nc()
    in_maps = make_in_maps(queries, keys, values, q_pos_embedding,
                           k_pos_embedding)
    res = run_bass_kernel_spmd(nc, in_maps, core_ids=list(range(B)))
    out = np.stack([r["o"].reshape(C, 64, 64) for r in res.results])
    return out.astype(np.float32)


def build_nc_trivial():
    """Same I/O signature, minimal work: used by test.py to subtract the
    per-call transfer/dispatch overhead from wall-clock timing."""
    nc = bacc.Bacc(None, target_bir_lowering=False)
    qt = nc.dram_tensor("qt", [C, N], BF16, kind="ExternalInput")
    kt = nc.dram_tensor("kt", [C, N], BF16, kind="ExternalInput")
    v = nc.dram_tensor("v", [N, C], BF16, kind="ExternalInput")
    qp = nc.dram_tensor("qp", [C, N], BF16, kind="ExternalInput")
    kp = nc.dram_tensor("kp", [C, N], BF16, kind="ExternalInput")
    o = nc.dram_tensor("o", [C, N], F32, kind="ExternalOutput")
    with tile.TileContext(nc) as tc:
        with tc.tile_pool(name="sb", bufs=2) as sb:
            t = sb.tile([P, 2, N], BF16, tag="t")
            nc.sync.dma_start(t, qt.rearrange("(co p) n -> p co n", p=P))
            nc.sync.dma_start(o.rearrange("(co p) n -> p co n", p=P), t)
    nc.compile()
    return nc


# revision 4
# speedup vs baseline: 1.2807x; 1.0026x over previous
"""Cross-attention kernel for Trainium2 (8 NeuronCores, batch-parallel).

Math per batch b (reference semantics):
  q = queries[b].reshape(C, N).T + q_pos        # [N, C]
  k = keys[b].reshape(C, N).T + k_pos
  v = values[b].reshape(C, N).T                 # [N, C]
  out = softmax(q @ k.T / 16) @ v, returned as [C, N] (c-major)

Device layout (per core = one batch):
  S is computed transposed (S^T[k, q]) so that exp(S^T) tiles are directly
  the rhs of the O matmul (O^T = V^T A^T) -- no on-chip transposes.

  S matmuls run as error-compensated fp8 (e4m3) in DoubleRow perf mode:
  q = q_hi + q_lo, k = k_hi + k_lo (hi = fp8 round, lo = fp8 residual) and
  S ~= Qh Kh + Ql Kh + Qh Kl (the dropped Ql Kl term is O(2^-8) relative).
  Each DoubleRow matmul contracts all 256 channels in one instruction.

  O matmuls use bf16 A (exp output) against bf16 V.  The softmax
  denominator is accumulated on the vector engine in fp16 (2x DVE mode)
  instead of burning PE streams on ones-matmuls per key chunk; a single
  ones-matmul per query block folds the 128 partitions.

  The hi/lo splits for the eight K blocks are due within query block 0;
  they alternate between DVE and GpSimd (Pool) so neither in-order queue
  stalls the first block's S matmuls.
"""

import numpy as np
import ml_dtypes

import concourse.bass as bass
import concourse.tile as tile
import concourse.mybir as mybir
from concourse import bacc
from concourse.bass_utils import run_bass_kernel_spmd

P = 128          # partitions
C = 256          # qk/v channel dim
N = 4096         # sequence (64*64)
B = 8            # batch == n_cores
QW = 512         # query block width (max fp32-class matmul free dim)
NQB = N // QW    # 8 query blocks
NKO = N // P     # 32 key chunks
KPB = QW // P    # key chunks per K block tile
SCALE = 1.0 / 16.0  # 1/sqrt(C)

F32 = mybir.dt.float32
F32R = mybir.dt.float32r
BF16 = mybir.dt.bfloat16
FP16 = mybir.dt.float16
FP8 = mybir.dt.float8e4
AF = mybir.ActivationFunctionType
DR = mybir.MatmulPerfMode.DoubleRow
MULT = mybir.AluOpType.mult
ADD = mybir.AluOpType.add

_NC_CACHE = None


def build_nc(atp_bufs=6, raw_bufs=3, ps_s_bufs=4, ps_o_bufs=1, lag=3):
    nc = bacc.Bacc(None, target_bir_lowering=False)
    qt = nc.dram_tensor("qt", [C, N], BF16, kind="ExternalInput")
    kt = nc.dram_tensor("kt", [C, N], BF16, kind="ExternalInput")
    v = nc.dram_tensor("v", [N, C], BF16, kind="ExternalInput")
    qp = nc.dram_tensor("qp", [C, N], BF16, kind="ExternalInput")
    kp = nc.dram_tensor("kp", [C, N], BF16, kind="ExternalInput")
    o = nc.dram_tensor("o", [C, N], F32, kind="ExternalOutput")

    qt3 = qt.rearrange("(co p) n -> p co n", p=P)
    kt3 = kt.rearrange("(co p) n -> p co n", p=P)
    qp3 = qp.rearrange("(co p) n -> p co n", p=P)
    kp3 = kp.rearrange("(co p) n -> p co n", p=P)
    v3 = v.rearrange("(ko p) c -> p ko c", p=P)

    with tile.TileContext(nc) as tc:
        with (
            tc.tile_pool(name="consts", bufs=1) as consts,
            tc.tile_pool(name="ksplit", bufs=NQB) as ksplit,
            tc.tile_pool(name="qsplit", bufs=3) as qsplit,
            tc.tile_pool(name="sums", bufs=2) as sums,
            tc.tile_pool(name="kraws", bufs=NQB) as kraws,
            tc.tile_pool(name="vp", bufs=NKO) as vp,
            tc.tile_pool(name="raw", bufs=raw_bufs) as raw,
            tc.tile_pool(name="atp", bufs=atp_bufs) as atp,
            tc.tile_pool(name="accp", bufs=2) as accp,
            tc.tile_pool(name="small", bufs=2) as small,
            tc.tile_pool(name="outp", bufs=2) as outp,
            tc.tile_pool(name="ps_s", bufs=ps_s_bufs, space="PSUM") as ps_s,
            tc.tile_pool(name="ps_o", bufs=ps_o_bufs, space="PSUM") as ps_o,
            tc.tile_pool(name="ps_r", bufs=1, space="PSUM") as ps_r,
            tc.tile_pool(name="ps_b", bufs=1, space="PSUM") as ps_b,
        ):
            ones_f = consts.tile([P, 2], F32, tag="ones_f")
            nc.vector.memset(ones_f, 1.0)
            ones_h = consts.tile([P, 2], FP16, tag="ones_h")
            nc.vector.tensor_copy(ones_h, ones_f)
            ones_rf = consts.tile([1, P], F32, tag="ones_rf")
            nc.vector.memset(ones_rf, 1.0)
            ones_r = consts.tile([1, P], F32R, tag="ones_r")
            nc.vector.tensor_copy(ones_r, ones_rf)

            # ---- K-block handling, split into DMA issue and engine prep ----
            def kblk_dma(j):
                sl = slice(j * QW, (j + 1) * QW)
                kraw = kraws.tile([P, 2, QW], BF16, tag="kraw")
                kpos = kraws.tile([P, 2, QW], BF16, tag="kpos")
                nc.sync.dma_start(kraw, kt3[:, :, sl])
                nc.sync.dma_start(kpos, kp3[:, :, sl])
                return kraw, kpos

            def kblk_prep(kraw, kpos, eng, split_co=False):
                ksum = sums.tile([P, 2, QW], BF16, tag="ksum")
                if split_co:
                    for co in range(2):
                        eng.tensor_add(ksum[:, co, :], kraw[:, co, :],
                                       kpos[:, co, :])
                else:
                    eng.tensor_add(ksum, kraw, kpos)
                khi = ksplit.tile([P, 2, QW], FP8, tag="khi")
                eng.tensor_copy(khi, ksum)
                klo = ksplit.tile([P, 2, QW], FP8, tag="klo")
                # klo = (khi * -1) + ksum
                eng.scalar_tensor_tensor(klo, khi, -1.0, ksum,
                                         op0=MULT, op1=ADD)
                return khi, klo

            def load_vchunk(ko):
                vc = vp.tile([P, C], BF16, tag="v")
                nc.sync.dma_start(vc, v3[:, ko, :])
                return vc

            def q_dma(j):
                sl = slice(j * QW, (j + 1) * QW)
                qraw = raw.tile([P, 2, QW], BF16, tag="qraw")
                qpos = raw.tile([P, 2, QW], BF16, tag="qpos")
                nc.sync.dma_start(qraw, qt3[:, :, sl])
                nc.sync.dma_start(qpos, qp3[:, :, sl])
                return qraw, qpos

            def q_prep(qraw, qpos, split_co=False):
                qsum = sums.tile([P, 2, QW], BF16, tag="qsum")
                if split_co:
                    for co in range(2):
                        nc.vector.tensor_add(qsum[:, co, :], qraw[:, co, :],
                                             qpos[:, co, :])
                else:
                    nc.vector.tensor_add(qsum, qraw, qpos)
                qhi = qsplit.tile([P, 2, QW], FP8, tag="qhi")
                nc.vector.tensor_copy(qhi, qsum)
                qlo = qsplit.tile([P, 2, QW], FP8, tag="qlo")
                nc.vector.scalar_tensor_tensor(qlo, qhi, -1.0, qsum,
                                               op0=MULT, op1=ADD)
                return qhi, qlo

            kblks = {}
            kraw_tiles = {}
            vcs = {}
            qdmas = {}

            # deadline-ordered DMA issue: K0, Q0, then V(4jb..) and K(jb+1)
            kraw_tiles[0] = kblk_dma(0)
            qdmas[0] = q_dma(0)
            kblks[0] = kblk_prep(*kraw_tiles[0], nc.vector, split_co=True)
            for jb in range(NQB):
                for ko in range(4 * jb, 4 * jb + 4):
                    vcs[ko] = load_vchunk(ko)
                if jb + 1 < NQB:
                    kraw_tiles[jb + 1] = kblk_dma(jb + 1)

            def emit_epilogue(j, po0, po1, pr, nslice=1):
                sl0 = j * QW
                sw = QW // nslice
                inv = small.tile([1, QW], F32R, tag="inv")
                pb = ps_b.tile([P, QW], F32, tag="b")
                bs = small.tile([P, QW], F32, tag="bs")
                oo0 = outp.tile([P, QW], F32, tag="oo0")
                oo1 = outp.tile([P, QW], F32, tag="oo1")
                for s in range(nslice):
                    ssl = slice(s * sw, (s + 1) * sw)
                    osl = slice(sl0 + s * sw, sl0 + (s + 1) * sw)
                    with nc.allow_low_precision(
                        reason="TF32 rounding of softmax reciprocal"
                    ):
                        nc.vector.reciprocal(inv[:, ssl], pr[0:1, ssl])
                    nc.tensor.matmul(pb[:, ssl], ones_r, inv[:, ssl],
                                     start=True, stop=True)
                    nc.vector.tensor_copy(bs[:, ssl], pb[:, ssl])
                    nc.vector.tensor_mul(oo0[:, ssl], po0[:, ssl], bs[:, ssl])
                    nc.sync.dma_start(o[0:P, osl], oo0[:, ssl])
                    nc.vector.tensor_mul(oo1[:, ssl], po1[:, ssl], bs[:, ssl])
                    nc.sync.dma_start(o[P:C, osl], oo1[:, ssl])

            pending = None
            qprepped = {0: q_prep(*qdmas[0], split_co=True)}
            for j in range(NQB):
                qhi, qlo = qprepped.pop(j)

                po0 = ps_o.tile([P, QW], F32, tag="o0")
                po1 = ps_o.tile([P, QW], F32, tag="o1")
                acc = accp.tile([P, QW], FP16, tag="acc")

                a_q = {}

                for ko in range(NKO):
                    jb, koff = divmod(ko, KPB)
                    if j == 0 and ko % KPB == 1 and jb + 1 < NQB:
                        # prep the next K block while this group computes;
                        # alternate engines so neither queue falls behind
                        eng = nc.vector if jb % 2 == 0 else nc.gpsimd
                        kblks[jb + 1] = kblk_prep(*kraw_tiles[jb + 1], eng)
                    if ko == 4 and j + 1 < NQB:
                        qdmas[j + 1] = q_dma(j + 1)
                    if ko == 6 and j + 1 < NQB:
                        qprepped[j + 1] = q_prep(*qdmas[j + 1])

                    pss = ps_s.tile([P, QW], F32, tag="s")
                    khi, klo = kblks[jb]
                    ksl = slice(koff * P, (koff + 1) * P)
                    nc.tensor.matmul(pss, khi[:, :, ksl], qhi,
                                     start=True, stop=False, perf_mode=DR)
                    nc.tensor.matmul(pss, khi[:, :, ksl], qlo,
                                     start=False, stop=False, perf_mode=DR)
                    nc.tensor.matmul(pss, klo[:, :, ksl], qhi,
                                     start=False, stop=True, perf_mode=DR)
                    a = atp.tile([P, QW], BF16, tag="a")
                    nc.scalar.activation(a, pss, AF.Exp, scale=SCALE)
                    a_q[ko] = a
                    # fp16 row-sum accumulator on DVE (2x mode: all 2-byte)
                    if ko == 0:
                        nc.vector.tensor_copy(acc, a)
                    else:
                        nc.vector.tensor_add(acc, acc, a)

                    if ko >= lag:
                        pko = ko - lag
                        av = a_q[pko]
                        nc.tensor.matmul(po0, vcs[pko][:, 0:P], av,
                                         start=(pko == 0), stop=False)
                        nc.tensor.matmul(po1, vcs[pko][:, P:C], av,
                                         start=(pko == 0), stop=False)
                        del a_q[pko]

                    if ko == 2 and pending is not None:
                        emit_epilogue(*pending)
                        pending = None

                # drain remaining lagged chunks; last closes the groups
                for pko in range(NKO - lag, NKO):
                    av = a_q[pko]
                    last = pko == NKO - 1
                    nc.tensor.matmul(po0, vcs[pko][:, 0:P], av,
                                     start=False, stop=last)
                    nc.tensor.matmul(po1, vcs[pko][:, P:C], av,
                                     start=False, stop=last)
                    del a_q[pko]

                # fold the 128 partitions of acc into the denominator row
                pr = ps_r.tile([2, QW], F32, tag="r")
                nc.tensor.matmul(pr, ones_h, acc, start=True, stop=True)
                pending = (j, po0, po1, pr)

            # last block: sliced epilogue to pipeline the serial tail
            emit_epilogue(*pending, nslice=4)

    nc.compile()
    return nc


def _get_nc():
    global _NC_CACHE
    if _NC_CACHE is None:
        _NC_CACHE = build_nc()
    return _NC_CACHE


def make_in_maps(queries, keys, values, q_pos_embedding, k_pos_embedding):
    bf16 = ml_dtypes.bfloat16
    queries = np.asarray(queries, dtype=np.float32)
    keys = np.asarray(keys, dtype=np.float32)
    values = np.asarray(values, dtype=np.float32)
    qpT = np.ascontiguousarray(
        np.asarray(q_pos_embedding, dtype=np.float32).reshape(N, C).T
    ).astype(bf16)
    kpT = np.ascontiguousarray(
        np.asarray(k_pos_embedding, dtype=np.float32).reshape(N, C).T
    ).astype(bf16)
    in_maps = []
    for b in range(B):
        vT = np.ascontiguousarray(values[b].reshape(C, N).T).astype(bf16)
        in_maps.append({
            "qt": np.ascontiguousarray(queries[b].reshape(C, N)).astype(bf16),
            "kt": np.ascontiguousarray(keys[b].reshape(C, N)).astype(bf16),
            "v": vT,
            "qp": qpT,
            "kp": kpT,
        })
    return in_maps


def kernel(queries, keys, values, q_pos_embedding, k_pos_embedding):
    nc = _get_nc()
    in_maps = make_in_maps(queries, keys, values, q_pos_embedding,
                           k_pos_embedding)
    res = run_bass_kernel_spmd(nc, in_maps, core_ids=list(range(B)))
    out = np.stack([r["o"].reshape(C, 64, 64) for r in res.results])
    return out.astype(np.float32)


def build_nc_trivial():
    """Same I/O signature, minimal work: used by test.py to subtract the
    per-call transfer/dispatch overhead from wall-clock timing."""
    nc = bacc.Bacc(None, target_bir_lowering=False)
    qt = nc.dram_tensor("qt", [C, N], BF16, kind="ExternalInput")
    kt = nc.dram_tensor("kt", [C, N], BF16, kind="ExternalInput")
    v = nc.dram_tensor("v", [N, C], BF16, kind="ExternalInput")
    qp = nc.dram_tensor("qp", [C, N], BF16, kind="ExternalInput")
    kp = nc.dram_tensor("kp", [C, N], BF16, kind="ExternalInput")
    o = nc.dram_tensor("o", [C, N], F32, kind="ExternalOutput")
    with tile.TileContext(nc) as tc:
        with tc.tile_pool(name="sb", bufs=2) as sb:
            t = sb.tile([P, 2, N], BF16, tag="t")
            nc.sync.dma_start(t, qt.rearrange("(co p) n -> p co n", p=P))
            nc.sync.dma_start(o.rearrange("(co p) n -> p co n", p=P), t)
    nc.compile()
    return nc


# revision 33
# speedup vs baseline: 1.3222x; 1.0324x over previous
"""Cross-attention kernel for Trainium2 (8 NeuronCores, batch-parallel).

Math per batch b (reference semantics):
  q = queries[b].reshape(C, N).T + q_pos        # [N, C]
  k = keys[b].reshape(C, N).T + k_pos
  v = values[b].reshape(C, N).T                 # [N, C]
  out = softmax(q @ k.T / 16) @ v, returned as [C, N] (c-major)

Device layout (per core = one batch):
  S is computed transposed (S^T[k, q]) so that exp(S^T) tiles are directly
  the rhs of the O matmul (O^T = V^T A^T) -- no on-chip transposes.

  S matmuls run as error-compensated fp8 (e4m3) in DoubleRow perf mode:
  q = q_hi + q_lo, k = k_hi + k_lo (hi = fp8 round, lo = fp8 residual) and
  S ~= Qh Kh + Ql Kh + Qh Kl (the dropped Ql Kl term is O(2^-8) relative).
  Each DoubleRow matmul contracts all 256 channels in one instruction.

  O matmuls use bf16 A (exp output) against bf16 V.  The softmax
  denominator is accumulated on the vector engine in fp16 (2x DVE mode)
  instead of burning PE streams on ones-matmuls per key chunk; a single
  ones-matmul per query block folds the 128 partitions.

  All five inputs live SBUF-resident, loaded by a handful of large DMAs
  (the cost model charges a fixed ~0.6us HWDGE occupancy per DMA, so many
  small transfers serialize badly).  The fp8 hi/lo splits of the eight
  K blocks are due within query block 0; they are pipelined across
  DVE (sum) -> Act (hi copy) -> Pool (lo residual) so no single in-order
  queue stalls the first block's S matmuls.
"""

import numpy as np
import ml_dtypes

import concourse.bass as bass
import concourse.tile as tile
import concourse.mybir as mybir
from concourse import bacc
from concourse.bass_utils import run_bass_kernel_spmd

P = 128          # partitions
C = 256          # qk/v channel dim
N = 4096         # sequence (64*64)
B = 8            # batch == n_cores
QW = 512         # query block width (max fp32-class matmul free dim)
NQB = N // QW    # 8 query blocks
NKO = N // P     # 32 key chunks
KPB = QW // P    # key chunks per K block tile
SCALE = 1.0 / 16.0  # 1/sqrt(C)

F32 = mybir.dt.float32
F32R = mybir.dt.float32r
BF16 = mybir.dt.bfloat16
FP16 = mybir.dt.float16
FP8 = mybir.dt.float8e4
AF = mybir.ActivationFunctionType
DR = mybir.MatmulPerfMode.DoubleRow
MULT = mybir.AluOpType.mult
ADD = mybir.AluOpType.add

_NC_CACHE = None


def build_nc(atp_bufs=8, ps_s_bufs=4, lag=4):
    nc = bacc.Bacc(None, target_bir_lowering=False)
    qt = nc.dram_tensor("qt", [C, N], BF16, kind="ExternalInput")
    kt = nc.dram_tensor("kt", [C, N], BF16, kind="ExternalInput")
    v = nc.dram_tensor("v", [N, C], BF16, kind="ExternalInput")
    # pos tables ride fp8: they are ~5% of q/k magnitude, so their fp8
    # quantization error lands ~2e-3 relative on the output; halves the
    # early DMA bytes, which bound the warmup
    qp = nc.dram_tensor("qp", [C, N], FP8, kind="ExternalInput")
    kp = nc.dram_tensor("kp", [C, N], FP8, kind="ExternalInput")
    o = nc.dram_tensor("o", [C, N], F32, kind="ExternalOutput")

    qt3 = qt.rearrange("(co p) n -> p co n", p=P)
    kt3 = kt.rearrange("(co p) n -> p co n", p=P)
    qp3 = qp.rearrange("(co p) n -> p co n", p=P)
    kp3 = kp.rearrange("(co p) n -> p co n", p=P)
    v3 = v.rearrange("(ko p) c -> p ko c", p=P)
    o3 = o.rearrange("(co p) n -> p co n", p=P)

    with tile.TileContext(nc) as tc:
        with (
            tc.tile_pool(name="consts", bufs=1) as consts,
            tc.tile_pool(name="inputs", bufs=1) as inputs,
            tc.tile_pool(name="ksplit", bufs=NQB) as ksplit,
            tc.tile_pool(name="qsplit", bufs=3) as qsplit,
            tc.tile_pool(name="ksums", bufs=NQB) as ksums,
            tc.tile_pool(name="sums", bufs=3) as sums,
            tc.tile_pool(name="atp", bufs=atp_bufs) as atp,
            tc.tile_pool(name="accp", bufs=2) as accp,
            tc.tile_pool(name="small", bufs=2) as small,
            tc.tile_pool(name="outp", bufs=2) as outp,
            tc.tile_pool(name="ps_s", bufs=ps_s_bufs, space="PSUM") as ps_s,
            tc.tile_pool(name="ps_o", bufs=1, space="PSUM") as ps_o,
            tc.tile_pool(name="ps_r", bufs=1, space="PSUM") as ps_r,
            tc.tile_pool(name="ps_b", bufs=1, space="PSUM") as ps_b,
        ):
            ones_f = consts.tile([P, 2], F32, tag="ones_f")
            nc.vector.memset(ones_f, 1.0)
            ones_h = consts.tile([P, 2], FP16, tag="ones_h")
            nc.vector.tensor_copy(ones_h, ones_f)
            ones_rf = consts.tile([1, P], F32, tag="ones_rf")
            nc.vector.memset(ones_rf, 1.0)
            ones_r = consts.tile([1, P], F32R, tag="ones_r")
            nc.vector.tensor_copy(ones_r, ones_rf)
            exp_bias = consts.tile([P, 1], F32, tag="exp_bias")
            nc.vector.memset(exp_bias, -7.0)


            # SBUF-resident inputs, loaded in deadline order.  The shared
            # DMA path is nearly saturated during query block 0 (all of K,
            # pos, V and the first Q blocks are due then), so transfers are
            # sliced to arrive just before their consumers.
            ktile = inputs.tile([P, 2, N], BF16, tag="ktile")
            kptile = inputs.tile([P, 2, N], FP8, tag="kptile")
            qtile = inputs.tile([P, 2, N], BF16, tag="qtile")
            qptile = inputs.tile([P, 2, N], FP8, tag="qptile")
            vtile = inputs.tile([P, NKO, C], BF16, tag="vtile")

            def blks(t3, dst, jlo, jhi):
                sl = slice(jlo * QW, jhi * QW)
                nc.sync.dma_start(dst[:, :, sl], t3[:, :, sl])

            blks(kt3, ktile, 0, 1)
            blks(kp3, kptile, 0, 1)
            blks(qt3, qtile, 0, 1)
            blks(qp3, qptile, 0, 1)
            nc.sync.dma_start(vtile[:, 0:4, :], v3[:, 0:4, :])
            blks(kt3, ktile, 1, 3)
            blks(qt3, qtile, 1, 3)
            blks(kp3, kptile, 1, NQB)
            blks(kt3, ktile, 3, 5)
            nc.sync.dma_start(vtile[:, 4:10, :], v3[:, 4:10, :])
            blks(qp3, qptile, 1, NQB)
            nc.sync.dma_start(vtile[:, 10:16, :], v3[:, 10:16, :])
            blks(kt3, ktile, 5, 7)
            nc.sync.dma_start(vtile[:, 16:24, :], v3[:, 16:24, :])
            blks(kt3, ktile, 7, 8)
            blks(qt3, qtile, 3, 5)
            nc.sync.dma_start(vtile[:, 24:NKO, :], v3[:, 24:NKO, :])
            blks(qt3, qtile, 5, 8)

            def ksum_make(j, eng, split_co=False):
                """K block j pos-add (bf16, retained for the fp8 split)."""
                sl = slice(j * QW, (j + 1) * QW)
                ksum = ksums.tile([P, 2, QW], BF16, tag="ksum")
                if split_co:
                    for co in range(2):
                        eng.tensor_add(ksum[:, co, :], ktile[:, co, sl],
                                       kptile[:, co, sl])
                else:
                    eng.tensor_add(ksum, ktile[:, :, sl], kptile[:, :, sl])
                return ksum

            def ksplit_make(ksum):
                """hi/lo fp8 split of a K block, spread Act -> Pool; only
                needed from query block 1 on (block 0 runs S in bf16)."""
                khi = ksplit.tile([P, 2, QW], FP8, tag="khi")
                nc.scalar.activation(khi, ksum, AF.Copy)
                klo = ksplit.tile([P, 2, QW], FP8, tag="klo")
                nc.gpsimd.tensor_sub(klo, ksum, khi)
                return khi, klo

            def q_prep(q0, w):
                sl = slice(q0, q0 + w)
                qsum = sums.tile([P, 2, QW], BF16, tag="qsum")
                nc.vector.tensor_add(qsum[:, :, 0:w], qtile[:, :, sl],
                                     qptile[:, :, sl])
                qhi = qsplit.tile([P, 2, QW], FP8, tag="qhi")
                nc.vector.tensor_copy(qhi[:, :, 0:w], qsum[:, :, 0:w])
                qlo = qsplit.tile([P, 2, QW], FP8, tag="qlo")
                nc.vector.tensor_sub(qlo[:, :, 0:w], qsum[:, :, 0:w],
                                     qhi[:, :, 0:w])
                return qhi, qlo

            ksum_tiles = {0: ksum_make(0, nc.vector, split_co=True)}
            kblks = {}

            def emit_epilogue(q0, w, po0, po1, pr, final=False):
                inv = small.tile([1, QW], F32R, tag="inv")
                with nc.allow_low_precision(
                    reason="TF32 rounding of softmax reciprocal"
                ):
                    nc.vector.reciprocal(inv[:, 0:w], pr[0:1, 0:w])
                pb = ps_b.tile([P, QW], F32, tag="b")
                nc.tensor.matmul(pb[:, 0:w], ones_r, inv[:, 0:w],
                                 start=True, stop=True)
                oo = outp.tile([P, 2, QW], F32, tag="oo")
                bs = small.tile([P, QW], F32, tag="bs")
                if final:
                    # tail latency: pipeline scale + writeback in halves
                    hw_ = w // 2
                    for h in range(2):
                        hs = slice(h * hw_, (h + 1) * hw_)
                        nc.vector.tensor_copy(bs[:, hs], pb[:, hs])
                        nc.vector.tensor_mul(oo[:, 0, hs], po0[:, hs],
                                             bs[:, hs])
                        nc.vector.tensor_mul(oo[:, 1, hs], po1[:, hs],
                                             bs[:, hs])
                        nc.sync.dma_start(
                            o3[:, :, q0 + h * hw_:q0 + (h + 1) * hw_],
                            oo[:, :, hs])
                else:
                    nc.vector.tensor_copy(bs[:, 0:w], pb[:, 0:w])
                    nc.vector.tensor_mul(oo[:, 0, 0:w], po0[:, 0:w],
                                         bs[:, 0:w])
                    nc.vector.tensor_mul(oo[:, 1, 0:w], po1[:, 0:w],
                                         bs[:, 0:w])
                    nc.sync.dma_start(o3[:, :, q0:q0 + w], oo[:, :, 0:w])

            pending = None
            # block 0 runs S in bf16 straight from the sums (no fp8 split):
            # its prep is one DVE add per K block, so the warmup is bounded
            # by DMA arrival, not by split chains
            qsum0 = sums.tile([P, 2, QW], BF16, tag="qsum0")
            for co in range(2):
                nc.vector.tensor_add(qsum0[:, co, :], qtile[:, co, 0:QW],
                                     qptile[:, co, 0:QW])
            qprepped = {}
            widths = [QW] * NQB
            starts = [sum(widths[:i]) for i in range(len(widths))]
            nblk = len(widths)
            for j in range(nblk):
                w = widths[j]
                q0 = starts[j]
                qhi, qlo = qprepped.pop(j) if j > 0 else (None, None)

                po0 = ps_o.tile([P, QW], F32, tag="o0")
                po1 = ps_o.tile([P, QW], F32, tag="o1")
                acc = accp.tile([P, QW], FP16, tag="acc")

                a_q = {}

                for ko in range(NKO):
                    jb, koff = divmod(ko, KPB)
                    if j == 0:
                        jt = ko // 2 + 1
                        if ko % 2 == 0 and jt < NQB:
                            # pos-add for K block jt (bf16, cheap, one op)
                            eng = nc.vector if jt < 5 else nc.gpsimd
                            ksum_tiles[jt] = ksum_make(jt, eng)
                        if ko >= 16 and ko % 2 == 0:
                            # fp8 hi/lo splits, due from block 1 on; Act
                            # and Pool do them so DVE keeps up with acc
                            js = (ko - 16) // 2
                            kblks[js] = ksplit_make(ksum_tiles[js])
                    if ko == 6 and j + 1 < nblk:
                        qprepped[j + 1] = q_prep(starts[j + 1],
                                                 widths[j + 1])

                    pss = ps_s.tile([P, QW], F32, tag="s")
                    ksl = slice(koff * P, (koff + 1) * P)
                    if j == 0 and ko >= 16 and jb in kblks \
                            and qprepped.get("q0") is not None:
                        khi, klo = kblks[jb]
                        q0hi, q0lo = qprepped["q0"]
                        nc.tensor.matmul(pss[:, 0:w], khi[:, :, ksl],
                                         q0hi[:, :, 0:w],
                                         start=True, stop=False, perf_mode=DR)
                        nc.tensor.matmul(pss[:, 0:w], khi[:, :, ksl],
                                         q0lo[:, :, 0:w],
                                         start=False, stop=False,
                                         perf_mode=DR)
                        nc.tensor.matmul(pss[:, 0:w], klo[:, :, ksl],
                                         q0hi[:, :, 0:w],
                                         start=False, stop=True, perf_mode=DR)
                    elif j == 0:
                        ksum = ksum_tiles[jb]
                        for co in range(2):
                            nc.tensor.matmul(pss[:, 0:w], ksum[:, co, ksl],
                                             qsum0[:, co, 0:w],
                                             start=(co == 0), stop=(co == 1))
                    else:
                        khi, klo = kblks[jb]
                        nc.tensor.matmul(pss[:, 0:w], khi[:, :, ksl],
                                         qhi[:, :, 0:w],
                                         start=True, stop=False, perf_mode=DR)
                        nc.tensor.matmul(pss[:, 0:w], khi[:, :, ksl],
                                         qlo[:, :, 0:w],
                                         start=False, stop=False,
                                         perf_mode=DR)
                        nc.tensor.matmul(pss[:, 0:w], klo[:, :, ksl],
                                         qhi[:, :, 0:w],
                                         start=False, stop=True, perf_mode=DR)
                    a = atp.tile([P, QW], BF16, tag="a")
                    # exp is biased by a constant (cancels in the softmax
                    # division): logits reach ~14, and an unshifted exp
                    # overflows the fp16 row-sum accumulator
                    nc.scalar.activation(a[:, 0:w], pss[:, 0:w], AF.Exp,
                                         scale=SCALE, bias=exp_bias)
                    a_q[ko] = a
                    # fp16 row-sum accumulator on DVE (2x mode: all 2-byte)
                    if ko == 0:
                        nc.vector.tensor_copy(acc[:, 0:w], a[:, 0:w])
                    else:
                        nc.vector.tensor_add(acc[:, 0:w], acc[:, 0:w],
                                             a[:, 0:w])

                    if ko >= lag:
                        pko = ko - lag
                        av = a_q[pko]
                        nc.tensor.matmul(po0[:, 0:w], vtile[:, pko, 0:P],
                                         av[:, 0:w],
                                         start=(pko == 0), stop=False)
                        nc.tensor.matmul(po1[:, 0:w], vtile[:, pko, P:C],
                                         av[:, 0:w],
                                         start=(pko == 0),
                                         stop=(pko == NKO - 1))
                        del a_q[pko]

                    if ko == 1 and pending is not None:
                        # fold the previous block's partition accumulator
                        # here: its DVE chain has settled, so the PE never
                        # stalls on it at the block boundary
                        pq0, pw, ppo0, ppo1, pacc = pending
                        pr = ps_r.tile([2, QW], F32, tag="r")
                        nc.tensor.matmul(pr[:, 0:pw], ones_h,
                                         pacc[:, 0:pw],
                                         start=True, stop=True)
                        pending = (pq0, pw, ppo0, ppo1, pr)
                    if ko == 2 and pending is not None:
                        emit_epilogue(*pending)
                        pending = None

                # drain remaining lagged chunks; last closes the groups
                final = j == nblk - 1
                for pko in range(NKO - lag, NKO):
                    av = a_q[pko]
                    last = pko == NKO - 1
                    if final and last:
                        # the final denominator fold goes ahead of the last
                        # O pair: acc is ready, so the epilogue chain
                        # (recip -> pb -> muls -> dma) starts sooner
                        pr = ps_r.tile([2, QW], F32, tag="r")
                        nc.tensor.matmul(pr[:, 0:w], ones_h, acc[:, 0:w],
                                         start=True, stop=True)
                    nc.tensor.matmul(po0[:, 0:w], vtile[:, pko, 0:P],
                                     av[:, 0:w], start=False, stop=last)
                    nc.tensor.matmul(po1[:, 0:w], vtile[:, pko, P:C],
                                     av[:, 0:w], start=False, stop=last)
                    del a_q[pko]

                pending = (q0, w, po0, po1, acc)

            pq0, pw, ppo0, ppo1, pacc = pending
            emit_epilogue(pq0, pw, ppo0, ppo1, pr, final=True)

    nc.compile()
    return nc


def _get_nc():
    global _NC_CACHE
    if _NC_CACHE is None:
        _NC_CACHE = build_nc()
    return _NC_CACHE


def make_in_maps(queries, keys, values, q_pos_embedding, k_pos_embedding):
    bf16 = ml_dtypes.bfloat16
    queries = np.asarray(queries, dtype=np.float32)
    keys = np.asarray(keys, dtype=np.float32)
    values = np.asarray(values, dtype=np.float32)
    fp8 = ml_dtypes.float8_e4m3
    qpT = np.ascontiguousarray(
        np.asarray(q_pos_embedding, dtype=np.float32).reshape(N, C).T
    ).astype(fp8)
    kpT = np.ascontiguousarray(
        np.asarray(k_pos_embedding, dtype=np.float32).reshape(N, C).T
    ).astype(fp8)
    in_maps = []
    for b in range(B):
        vT = np.ascontiguousarray(values[b].reshape(C, N).T).astype(bf16)
        in_maps.append({
            "qt": np.ascontiguousarray(queries[b].reshape(C, N)).astype(bf16),
            "kt": np.ascontiguousarray(keys[b].reshape(C, N)).astype(bf16),
            "v": vT,
            "qp": qpT,
            "kp": kpT,
        })
    return in_maps


def kernel(queries, keys, values, q_pos_embedding, k_pos_embedding):
    nc = _get_nc()
    in_maps = make_in_maps(queries, keys, values, q_pos_embedding,
                           k_pos_embedding)
    res = run_bass_kernel_spmd(nc, in_maps, core_ids=list(range(B)))
    out = np.stack([r["o"].reshape(C, 64, 64) for r in res.results])
    return out.astype(np.float32)


def build_nc_trivial():
    """Same I/O signature, minimal work: used by test.py to subtract the
    per-call transfer/dispatch overhead from wall-clock timing."""
    nc = bacc.Bacc(None, target_bir_lowering=False)
    qt = nc.dram_tensor("qt", [C, N], BF16, kind="ExternalInput")
    kt = nc.dram_tensor("kt", [C, N], BF16, kind="ExternalInput")
    v = nc.dram_tensor("v", [N, C], BF16, kind="ExternalInput")
    qp = nc.dram_tensor("qp", [C, N], BF16, kind="ExternalInput")
    kp = nc.dram_tensor("kp", [C, N], BF16, kind="ExternalInput")
    o = nc.dram_tensor("o", [C, N], F32, kind="ExternalOutput")
    with tile.TileContext(nc) as tc:
        with tc.tile_pool(name="sb", bufs=2) as sb:
            t = sb.tile([P, 2, N], BF16, tag="t")
            nc.sync.dma_start(t, qt.rearrange("(co p) n -> p co n", p=P))
            nc.sync.dma_start(o.rearrange("(co p) n -> p co n", p=P), t)
    nc.compile()
    return nc


# revision 38
# speedup vs baseline: 1.3414x; 1.0145x over previous
"""Cross-attention kernel for Trainium2 (8 NeuronCores, batch-parallel).

Math per batch b (reference semantics):
  q = queries[b].reshape(C, N).T + q_pos        # [N, C]
  k = keys[b].reshape(C, N).T + k_pos
  v = values[b].reshape(C, N).T                 # [N, C]
  out = softmax(q @ k.T / 16) @ v, returned as [C, N] (c-major)

Device layout (per core = one batch):
  S is computed transposed (S^T[k, q]) so that exp(S^T) tiles are directly
  the rhs of the O matmul (O^T = V^T A^T) -- no on-chip transposes.

  S matmuls run as error-compensated fp8 (e4m3) in DoubleRow perf mode:
  q = q_hi + q_lo, k = k_hi + k_lo (hi = fp8 round, lo = fp8 residual) and
  S ~= Qh Kh + Ql Kh + Qh Kl (the dropped Ql Kl term is O(2^-8) relative).
  Each DoubleRow matmul contracts all 256 channels in one instruction.

  O matmuls use bf16 A (exp output) against bf16 V.  The softmax
  denominator is accumulated on the vector engine in fp16 (2x DVE mode)
  instead of burning PE streams on ones-matmuls per key chunk; a single
  ones-matmul per query block folds the 128 partitions.

  All five inputs live SBUF-resident, loaded by a handful of large DMAs
  (the cost model charges a fixed ~0.6us HWDGE occupancy per DMA, so many
  small transfers serialize badly).  The fp8 hi/lo splits of the eight
  K blocks are due within query block 0; they are pipelined across
  DVE (sum) -> Act (hi copy) -> Pool (lo residual) so no single in-order
  queue stalls the first block's S matmuls.
"""

import numpy as np
import ml_dtypes

import concourse.bass as bass
import concourse.tile as tile
import concourse.mybir as mybir
from concourse import bacc
from concourse.bass_utils import run_bass_kernel_spmd

P = 128          # partitions
C = 256          # qk/v channel dim
N = 4096         # sequence (64*64)
B = 8            # batch == n_cores
QW = 512         # query block width (max fp32-class matmul free dim)
NQB = N // QW    # 8 query blocks
NKO = N // P     # 32 key chunks
KPB = QW // P    # key chunks per K block tile
SCALE = 1.0 / 16.0  # 1/sqrt(C)

F32 = mybir.dt.float32
F32R = mybir.dt.float32r
BF16 = mybir.dt.bfloat16
FP16 = mybir.dt.float16
FP8 = mybir.dt.float8e4
AF = mybir.ActivationFunctionType
DR = mybir.MatmulPerfMode.DoubleRow
MULT = mybir.AluOpType.mult
ADD = mybir.AluOpType.add

_NC_CACHE = None


def build_nc(atp_bufs=12, ps_s_bufs=5, lag=6):
    nc = bacc.Bacc(None, target_bir_lowering=False)
    qt = nc.dram_tensor("qt", [C, N], BF16, kind="ExternalInput")
    kt = nc.dram_tensor("kt", [C, N], BF16, kind="ExternalInput")
    v = nc.dram_tensor("v", [N, C], BF16, kind="ExternalInput")
    # pos tables ride fp8: they are ~5% of q/k magnitude, so their fp8
    # quantization error lands ~2e-3 relative on the output; halves the
    # early DMA bytes, which bound the warmup
    qp = nc.dram_tensor("qp", [C, N], FP8, kind="ExternalInput")
    kp = nc.dram_tensor("kp", [C, N], FP8, kind="ExternalInput")
    o = nc.dram_tensor("o", [C, N], F32, kind="ExternalOutput")

    qt3 = qt.rearrange("(co p) n -> p co n", p=P)
    kt3 = kt.rearrange("(co p) n -> p co n", p=P)
    qp3 = qp.rearrange("(co p) n -> p co n", p=P)
    kp3 = kp.rearrange("(co p) n -> p co n", p=P)
    v3 = v.rearrange("(ko p) c -> p ko c", p=P)
    o3 = o.rearrange("(co p) n -> p co n", p=P)

    with tile.TileContext(nc) as tc:
        with (
            tc.tile_pool(name="consts", bufs=1) as consts,
            tc.tile_pool(name="inputs", bufs=1) as inputs,
            tc.tile_pool(name="ksplit", bufs=NQB) as ksplit,
            tc.tile_pool(name="qsplit", bufs=3) as qsplit,
            tc.tile_pool(name="ksums", bufs=NQB) as ksums,
            tc.tile_pool(name="sums", bufs=3) as sums,
            tc.tile_pool(name="atp", bufs=atp_bufs) as atp,
            tc.tile_pool(name="accp", bufs=2) as accp,
            tc.tile_pool(name="small", bufs=2) as small,
            tc.tile_pool(name="outp", bufs=2) as outp,
            tc.tile_pool(name="ps_s", bufs=ps_s_bufs, space="PSUM") as ps_s,
            tc.tile_pool(name="ps_o", bufs=1, space="PSUM") as ps_o,
            tc.tile_pool(name="ps_b", bufs=1, space="PSUM") as ps_b,
        ):
            ones_f = consts.tile([P, 2], F32, tag="ones_f")
            nc.vector.memset(ones_f, 1.0)
            ones_h = consts.tile([P, 2], FP16, tag="ones_h")
            nc.vector.tensor_copy(ones_h, ones_f)
            ones_rf = consts.tile([1, P], F32, tag="ones_rf")
            nc.vector.memset(ones_rf, 1.0)
            ones_r = consts.tile([1, P], F32R, tag="ones_r")
            nc.vector.tensor_copy(ones_r, ones_rf)
            exp_bias = consts.tile([P, 1], F32, tag="exp_bias")
            nc.vector.memset(exp_bias, -7.0)


            # SBUF-resident inputs, loaded in deadline order.  The shared
            # DMA path is nearly saturated during query block 0 (all of K,
            # pos, V and the first Q blocks are due then), so transfers are
            # sliced to arrive just before their consumers.
            ktile = inputs.tile([P, 2, N], BF16, tag="ktile")
            kptile = inputs.tile([P, 2, N], FP8, tag="kptile")
            qtile = inputs.tile([P, 2, N], BF16, tag="qtile")
            qptile = inputs.tile([P, 2, N], FP8, tag="qptile")
            vtile = inputs.tile([P, NKO, C], BF16, tag="vtile")

            def blks(t3, dst, jlo, jhi):
                sl = slice(jlo * QW, jhi * QW)
                nc.sync.dma_start(dst[:, :, sl], t3[:, :, sl])

            blks(kt3, ktile, 0, 1)
            blks(kp3, kptile, 0, 1)
            blks(qt3, qtile, 0, 1)
            blks(qp3, qptile, 0, 1)
            nc.sync.dma_start(vtile[:, 0:4, :], v3[:, 0:4, :])
            blks(kt3, ktile, 1, 3)
            blks(qt3, qtile, 1, 3)
            blks(kp3, kptile, 1, NQB)
            blks(kt3, ktile, 3, 5)
            nc.sync.dma_start(vtile[:, 4:10, :], v3[:, 4:10, :])
            blks(qp3, qptile, 1, NQB)
            nc.sync.dma_start(vtile[:, 10:16, :], v3[:, 10:16, :])
            blks(kt3, ktile, 5, 7)
            nc.sync.dma_start(vtile[:, 16:24, :], v3[:, 16:24, :])
            blks(kt3, ktile, 7, 8)
            blks(qt3, qtile, 3, 5)
            nc.sync.dma_start(vtile[:, 24:NKO, :], v3[:, 24:NKO, :])
            blks(qt3, qtile, 5, 8)

            def ksum_make(j, eng, split_co=False):
                """K block j pos-add (bf16, retained for the fp8 split)."""
                sl = slice(j * QW, (j + 1) * QW)
                ksum = ksums.tile([P, 2, QW], BF16, tag="ksum")
                if split_co:
                    for co in range(2):
                        eng.tensor_add(ksum[:, co, :], ktile[:, co, sl],
                                       kptile[:, co, sl])
                else:
                    eng.tensor_add(ksum, ktile[:, :, sl], kptile[:, :, sl])
                return ksum

            def ksplit_make(ksum, on_act=True):
                """hi/lo fp8 split of a K block; the hi copy runs on Act or
                Pool, the lo residual always on Pool, so DVE stays free for
                the row-sum accumulator."""
                khi = ksplit.tile([P, 2, QW], FP8, tag="khi")
                if on_act:
                    nc.scalar.activation(khi, ksum, AF.Copy)
                else:
                    nc.gpsimd.tensor_copy(khi, ksum)
                klo = ksplit.tile([P, 2, QW], FP8, tag="klo")
                nc.gpsimd.tensor_sub(klo, ksum, khi)
                return khi, klo

            def q_prep(q0, w):
                sl = slice(q0, q0 + w)
                qsum = sums.tile([P, 2, QW], BF16, tag="qsum")
                nc.vector.tensor_add(qsum[:, :, 0:w], qtile[:, :, sl],
                                     qptile[:, :, sl])
                qhi = qsplit.tile([P, 2, QW], FP8, tag="qhi")
                nc.vector.tensor_copy(qhi[:, :, 0:w], qsum[:, :, 0:w])
                qlo = qsplit.tile([P, 2, QW], FP8, tag="qlo")
                nc.vector.tensor_sub(qlo[:, :, 0:w], qsum[:, :, 0:w],
                                     qhi[:, :, 0:w])
                return qhi, qlo

            ksum_tiles = {0: ksum_make(0, nc.vector, split_co=True)}
            kblks = {}

            def emit_epilogue(q0, w, po0, po1, pr, final=False):
                inv = small.tile([1, QW], F32R, tag="inv")
                with nc.allow_low_precision(
                    reason="TF32 rounding of softmax reciprocal"
                ):
                    nc.vector.reciprocal(inv[:, 0:w], pr[0:1, 0:w])
                pb = ps_b.tile([P, QW], F32, tag="b")
                nc.tensor.matmul(pb[:, 0:w], ones_r, inv[:, 0:w],
                                 start=True, stop=True)
                oo = outp.tile([P, 2, QW], F32, tag="oo")
                bs = small.tile([P, QW], F32, tag="bs")
                if final:
                    # tail latency: pipeline scale + writeback in halves
                    hw_ = w // 2
                    for h in range(2):
                        hs = slice(h * hw_, (h + 1) * hw_)
                        nc.vector.tensor_copy(bs[:, hs], pb[:, hs])
                        nc.vector.tensor_mul(oo[:, 0, hs], po0[:, hs],
                                             bs[:, hs])
                        nc.vector.tensor_mul(oo[:, 1, hs], po1[:, hs],
                                             bs[:, hs])
                        nc.sync.dma_start(
                            o3[:, :, q0 + h * hw_:q0 + (h + 1) * hw_],
                            oo[:, :, hs])
                else:
                    nc.vector.tensor_copy(bs[:, 0:w], pb[:, 0:w])
                    nc.vector.tensor_mul(oo[:, 0, 0:w], po0[:, 0:w],
                                         bs[:, 0:w])
                    nc.vector.tensor_mul(oo[:, 1, 0:w], po1[:, 0:w],
                                         bs[:, 0:w])
                    nc.sync.dma_start(o3[:, :, q0:q0 + w], oo[:, :, 0:w])

            pending = None
            # block 0 runs S in bf16 straight from the sums (no fp8 split):
            # its prep is one DVE add per K block, so the warmup is bounded
            # by DMA arrival, not by split chains
            qsum0 = sums.tile([P, 2, QW], BF16, tag="qsum0")
            for co in range(2):
                nc.vector.tensor_add(qsum0[:, co, :], qtile[:, co, 0:QW],
                                     qptile[:, co, 0:QW])
            qprepped = {}
            widths = [QW] * NQB
            starts = [sum(widths[:i]) for i in range(len(widths))]
            nblk = len(widths)
            for j in range(nblk):
                w = widths[j]
                q0 = starts[j]
                qhi, qlo = qprepped.pop(j) if j > 0 else (None, None)

                po0 = ps_o.tile([P, QW], F32, tag="o0")
                po1 = ps_o.tile([P, QW], F32, tag="o1")
                acc = accp.tile([P, QW], FP16, tag="acc")

                a_q = {}

                for ko in range(NKO):
                    jb, koff = divmod(ko, KPB)
                    if j == 0:
                        jt = ko // 2 + 1
                        if ko % 2 == 0 and jt < NQB:
                            # pos-add for K block jt (bf16, cheap, one op)
                            eng = nc.vector if jt < 5 else nc.gpsimd
                            ksum_tiles[jt] = ksum_make(jt, eng)
                        if ko >= 16 and ko % 2 == 0:
                            # fp8 hi/lo splits, due from block 1 on; Act
                            # and Pool do them so DVE keeps up with acc
                            js = (ko - 16) // 2
                            kblks[js] = ksplit_make(ksum_tiles[js])
                    if ko == 6 and j + 1 < nblk:
                        qprepped[j + 1] = q_prep(starts[j + 1],
                                                 widths[j + 1])

                    pss = ps_s.tile([P, QW], F32, tag="s")
                    ksl = slice(koff * P, (koff + 1) * P)
                    if j == 0:
                        ksum = ksum_tiles[jb]
                        for co in range(2):
                            nc.tensor.matmul(pss[:, 0:w], ksum[:, co, ksl],
                                             qsum0[:, co, 0:w],
                                             start=(co == 0), stop=(co == 1))
                    else:
                        khi, klo = kblks[jb]
                        nc.tensor.matmul(pss[:, 0:w], khi[:, :, ksl],
                                         qhi[:, :, 0:w],
                                         start=True, stop=False, perf_mode=DR)
                        nc.tensor.matmul(pss[:, 0:w], khi[:, :, ksl],
                                         qlo[:, :, 0:w],
                                         start=False, stop=False,
                                         perf_mode=DR)
                        nc.tensor.matmul(pss[:, 0:w], klo[:, :, ksl],
                                         qhi[:, :, 0:w],
                                         start=False, stop=True, perf_mode=DR)
                    a = atp.tile([P, QW], BF16, tag="a")
                    # exp is biased by a constant (cancels in the softmax
                    # division): logits reach ~14, and an unshifted exp
                    # overflows the fp16 row-sum accumulator
                    nc.scalar.activation(a[:, 0:w], pss[:, 0:w], AF.Exp,
                                         scale=SCALE, bias=exp_bias)
                    a_q[ko] = a
                    # fp16 row-sum accumulator on DVE (2x mode: all 2-byte)
                    if ko == 0:
                        nc.vector.tensor_copy(acc[:, 0:w], a[:, 0:w])
                    else:
                        nc.vector.tensor_add(acc[:, 0:w], acc[:, 0:w],
                                             a[:, 0:w])

                    if ko >= lag:
                        pko = ko - lag
                        av = a_q[pko]
                        nc.tensor.matmul(po0[:, 0:w], vtile[:, pko, 0:P],
                                         av[:, 0:w],
                                         start=(pko == 0), stop=False)
                        nc.tensor.matmul(po1[:, 0:w], vtile[:, pko, P:C],
                                         av[:, 0:w],
                                         start=(pko == 0),
                                         stop=(pko == NKO - 1))
                        del a_q[pko]

                    if ko == 1 and pending is not None:
                        # fold the previous block's partition accumulator
                        # here: its DVE chain has settled, so the PE never
                        # stalls on it at the block boundary
                        pq0, pw, ppo0, ppo1, pacc = pending
                        pr = ps_b.tile([P, QW], F32, tag="b")
                        nc.tensor.matmul(pr[0:2, 0:pw], ones_h,
                                         pacc[:, 0:pw],
                                         start=True, stop=True)
                        pending = (pq0, pw, ppo0, ppo1, pr)
                    if ko == 2 and pending is not None:
                        emit_epilogue(*pending)
                        pending = None

                # drain remaining lagged chunks; last closes the groups
                final = j == nblk - 1
                for pko in range(NKO - lag, NKO):
                    av = a_q[pko]
                    last = pko == NKO - 1
                    if final and last:
                        # the final denominator fold goes ahead of the last
                        # O pair: acc is ready, so the epilogue chain
                        # (recip -> pb -> muls -> dma) starts sooner
                        pr = ps_b.tile([P, QW], F32, tag="b")
                        nc.tensor.matmul(pr[0:2, 0:w], ones_h, acc[:, 0:w],
                                         start=True, stop=True)
                    nc.tensor.matmul(po0[:, 0:w], vtile[:, pko, 0:P],
                                     av[:, 0:w], start=False, stop=last)
                    nc.tensor.matmul(po1[:, 0:w], vtile[:, pko, P:C],
                                     av[:, 0:w], start=False, stop=last)
                    del a_q[pko]

                pending = (q0, w, po0, po1, acc)

            pq0, pw, ppo0, ppo1, pacc = pending
            emit_epilogue(pq0, pw, ppo0, ppo1, pr, final=True)

    nc.compile()
    return nc


def _get_nc():
    global _NC_CACHE
    if _NC_CACHE is None:
        _NC_CACHE = build_nc()
    return _NC_CACHE


def make_in_maps(queries, keys, values, q_pos_embedding, k_pos_embedding):
    bf16 = ml_dtypes.bfloat16
    queries = np.asarray(queries, dtype=np.float32)
    keys = np.asarray(keys, dtype=np.float32)
    values = np.asarray(values, dtype=np.float32)
    fp8 = ml_dtypes.float8_e4m3
    qpT = np.ascontiguousarray(
        np.asarray(q_pos_embedding, dtype=np.float32).reshape(N, C).T
    ).astype(fp8)
    kpT = np.ascontiguousarray(
        np.asarray(k_pos_embedding, dtype=np.float32).reshape(N, C).T
    ).astype(fp8)
    in_maps = []
    for b in range(B):
        vT = np.ascontiguousarray(values[b].reshape(C, N).T).astype(bf16)
        in_maps.append({
            "qt": np.ascontiguousarray(queries[b].reshape(C, N)).astype(bf16),
            "kt": np.ascontiguousarray(keys[b].reshape(C, N)).astype(bf16),
            "v": vT,
            "qp": qpT,
            "kp": kpT,
        })
    return in_maps


def kernel(queries, keys, values, q_pos_embedding, k_pos_embedding):
    nc = _get_nc()
    in_maps = make_in_maps(queries, keys, values, q_pos_embedding,
                           k_pos_embedding)
    res = run_bass_kernel_spmd(nc, in_maps, core_ids=list(range(B)))
    out = np.stack([r["o"].reshape(C, 64, 64) for r in res.results])
    return out.astype(np.float32)


def build_nc_trivial():
    """Same I/O signature, minimal work: used by test.py to subtract the
    per-call transfer/dispatch overhead from wall-clock timing."""
    nc = bacc.Bacc(None, target_bir_lowering=False)
    qt = nc.dram_tensor("qt", [C, N], BF16, kind="ExternalInput")
    kt = nc.dram_tensor("kt", [C, N], BF16, kind="ExternalInput")
    v = nc.dram_tensor("v", [N, C], BF16, kind="ExternalInput")
    qp = nc.dram_tensor("qp", [C, N], BF16, kind="ExternalInput")
    kp = nc.dram_tensor("kp", [C, N], BF16, kind="ExternalInput")
    o = nc.dram_tensor("o", [C, N], F32, kind="ExternalOutput")
    with tile.TileContext(nc) as tc:
        with tc.tile_pool(name="sb", bufs=2) as sb:
            t = sb.tile([P, 2, N], BF16, tag="t")
            nc.sync.dma_start(t, qt.rearrange("(co p) n -> p co n", p=P))
            nc.sync.dma_start(o.rearrange("(co p) n -> p co n", p=P), t)
    nc.compile()
    return nc


# revision 49
# speedup vs baseline: 1.3559x; 1.0108x over previous
"""Cross-attention kernel for Trainium2 (8 NeuronCores, batch-parallel).

Math per batch b (reference semantics):
  q = queries[b].reshape(C, N).T + q_pos        # [N, C]
  k = keys[b].reshape(C, N).T + k_pos
  v = values[b].reshape(C, N).T                 # [N, C]
  out = softmax(q @ k.T / 16) @ v, returned as [C, N] (c-major)

Device layout (per core = one batch):
  S is computed transposed (S^T[k, q]) so that exp(S^T) tiles are directly
  the rhs of the O matmul (O^T = V^T A^T) -- no on-chip transposes.

  S matmuls run as error-compensated fp8 (e4m3) in DoubleRow perf mode:
  q = q_hi + q_lo, k = k_hi + k_lo (hi = fp8 round, lo = fp8 residual) and
  S ~= Qh Kh + Ql Kh + Qh Kl (the dropped Ql Kl term is O(2^-8) relative).
  Each DoubleRow matmul contracts all 256 channels in one instruction.

  O matmuls use bf16 A (exp output) against bf16 V.  The softmax
  denominator is accumulated on the vector engine in fp16 (2x DVE mode)
  instead of burning PE streams on ones-matmuls per key chunk; a single
  ones-matmul per query block folds the 128 partitions.

  All five inputs live SBUF-resident, loaded by a handful of large DMAs
  (the cost model charges a fixed ~0.6us HWDGE occupancy per DMA, so many
  small transfers serialize badly).  The fp8 hi/lo splits of the eight
  K blocks are due within query block 0; they are pipelined across
  DVE (sum) -> Act (hi copy) -> Pool (lo residual) so no single in-order
  queue stalls the first block's S matmuls.
"""

import numpy as np
import ml_dtypes

import concourse.bass as bass
import concourse.tile as tile
import concourse.mybir as mybir
from concourse import bacc
from concourse.bass_utils import run_bass_kernel_spmd

P = 128          # partitions
C = 256          # qk/v channel dim
N = 4096         # sequence (64*64)
B = 8            # batch == n_cores
QW = 512         # query block width (max fp32-class matmul free dim)
NQB = N // QW    # 8 query blocks
NKO = N // P     # 32 key chunks
KPB = QW // P    # key chunks per K block tile
SCALE = 1.0 / 16.0  # 1/sqrt(C)

F32 = mybir.dt.float32
F32R = mybir.dt.float32r
BF16 = mybir.dt.bfloat16
FP16 = mybir.dt.float16
FP8 = mybir.dt.float8e4
AF = mybir.ActivationFunctionType
DR = mybir.MatmulPerfMode.DoubleRow
MULT = mybir.AluOpType.mult
ADD = mybir.AluOpType.add

_NC_CACHE = None


def build_nc(atp_bufs=12, ps_s_bufs=5, lag=6):
    nc = bacc.Bacc(None, target_bir_lowering=False)
    qt = nc.dram_tensor("qt", [C, N], BF16, kind="ExternalInput")
    kt = nc.dram_tensor("kt", [C, N], BF16, kind="ExternalInput")
    v = nc.dram_tensor("v", [N, C], BF16, kind="ExternalInput")
    # pos tables ride fp8: they are ~5% of q/k magnitude, so their fp8
    # quantization error lands ~2e-3 relative on the output; halves the
    # early DMA bytes, which bound the warmup
    qp = nc.dram_tensor("qp", [C, N], FP8, kind="ExternalInput")
    kp = nc.dram_tensor("kp", [C, N], FP8, kind="ExternalInput")
    o = nc.dram_tensor("o", [C, N], F32, kind="ExternalOutput")

    qt3 = qt.rearrange("(co p) n -> p co n", p=P)
    kt3 = kt.rearrange("(co p) n -> p co n", p=P)
    qp3 = qp.rearrange("(co p) n -> p co n", p=P)
    kp3 = kp.rearrange("(co p) n -> p co n", p=P)
    v3 = v.rearrange("(ko p) c -> p ko c", p=P)
    o3 = o.rearrange("(co p) n -> p co n", p=P)

    with tile.TileContext(nc) as tc:
        with (
            tc.tile_pool(name="consts", bufs=1) as consts,
            tc.tile_pool(name="inputs", bufs=1) as inputs,
            tc.tile_pool(name="ksplit", bufs=NQB) as ksplit,
            tc.tile_pool(name="qsplit", bufs=3) as qsplit,
            tc.tile_pool(name="ksums", bufs=NQB) as ksums,
            tc.tile_pool(name="sums", bufs=3) as sums,
            tc.tile_pool(name="atp", bufs=atp_bufs) as atp,
            tc.tile_pool(name="accp", bufs=2) as accp,
            tc.tile_pool(name="small", bufs=2) as small,
            tc.tile_pool(name="outp", bufs=2) as outp,
            tc.tile_pool(name="ps_s", bufs=ps_s_bufs, space="PSUM") as ps_s,
            tc.tile_pool(name="ps_o", bufs=1, space="PSUM") as ps_o,
            tc.tile_pool(name="ps_b", bufs=1, space="PSUM") as ps_b,
        ):
            ones_f = consts.tile([P, 2], F32, tag="ones_f")
            nc.vector.memset(ones_f, 1.0)
            ones_h = consts.tile([P, 2], FP16, tag="ones_h")
            nc.vector.tensor_copy(ones_h, ones_f)
            ones_rf = consts.tile([1, P], F32, tag="ones_rf")
            nc.vector.memset(ones_rf, 1.0)
            ones_r = consts.tile([1, P], F32R, tag="ones_r")
            nc.vector.tensor_copy(ones_r, ones_rf)
            exp_bias = consts.tile([P, 1], F32, tag="exp_bias")
            nc.vector.memset(exp_bias, -7.0)


            # SBUF-resident inputs, loaded in deadline order.  The shared
            # DMA path is nearly saturated during query block 0 (all of K,
            # pos, V and the first Q blocks are due then), so transfers are
            # sliced to arrive just before their consumers.
            ktile = inputs.tile([P, 2, N], BF16, tag="ktile")
            kptile = inputs.tile([P, 2, N], FP8, tag="kptile")
            qtile = inputs.tile([P, 2, N], BF16, tag="qtile")
            qptile = inputs.tile([P, 2, N], FP8, tag="qptile")
            vtile = inputs.tile([P, NKO, C], BF16, tag="vtile")

            def blks(t3, dst, jlo, jhi):
                sl = slice(jlo * QW, jhi * QW)
                nc.sync.dma_start(dst[:, :, sl], t3[:, :, sl])

            blks(kt3, ktile, 0, 1)
            blks(kp3, kptile, 0, 1)
            blks(qt3, qtile, 0, 1)
            blks(qp3, qptile, 0, 1)
            nc.sync.dma_start(vtile[:, 0:4, :], v3[:, 0:4, :])
            blks(kt3, ktile, 1, 3)
            blks(kp3, kptile, 1, 4)
            blks(qt3, qtile, 1, 3)
            blks(kt3, ktile, 3, 5)
            nc.sync.dma_start(vtile[:, 4:10, :], v3[:, 4:10, :])
            blks(kp3, kptile, 4, NQB)
            blks(qp3, qptile, 1, 4)
            blks(qp3, qptile, 4, NQB)
            nc.sync.dma_start(vtile[:, 10:16, :], v3[:, 10:16, :])
            blks(kt3, ktile, 5, 7)
            nc.sync.dma_start(vtile[:, 16:20, :], v3[:, 16:20, :])
            nc.sync.dma_start(vtile[:, 20:24, :], v3[:, 20:24, :])
            blks(kt3, ktile, 7, 8)
            blks(qt3, qtile, 3, 5)
            nc.sync.dma_start(vtile[:, 24:NKO, :], v3[:, 24:NKO, :])
            blks(qt3, qtile, 5, 8)

            def ksum_make(j, eng, split_co=False):
                """K block j pos-add (bf16, retained for the fp8 split)."""
                sl = slice(j * QW, (j + 1) * QW)
                ksum = ksums.tile([P, 2, QW], BF16, tag="ksum")
                if split_co:
                    for co in range(2):
                        eng.tensor_add(ksum[:, co, :], ktile[:, co, sl],
                                       kptile[:, co, sl])
                else:
                    eng.tensor_add(ksum, ktile[:, :, sl], kptile[:, :, sl])
                return ksum

            def ksplit_make(ksum, on_act=True):
                """hi/lo fp8 split of a K block; the hi copy runs on Act or
                Pool, the lo residual always on Pool, so DVE stays free for
                the row-sum accumulator."""
                khi = ksplit.tile([P, 2, QW], FP8, tag="khi")
                if on_act:
                    nc.scalar.activation(khi, ksum, AF.Copy)
                else:
                    nc.gpsimd.tensor_copy(khi, ksum)
                klo = ksplit.tile([P, 2, QW], FP8, tag="klo")
                nc.gpsimd.tensor_sub(klo, ksum, khi)
                return khi, klo

            def q_prep(q0, w):
                sl = slice(q0, q0 + w)
                qsum = sums.tile([P, 2, QW], BF16, tag="qsum")
                nc.vector.tensor_add(qsum[:, :, 0:w], qtile[:, :, sl],
                                     qptile[:, :, sl])
                qhi = qsplit.tile([P, 2, QW], FP8, tag="qhi")
                nc.vector.tensor_copy(qhi[:, :, 0:w], qsum[:, :, 0:w])
                qlo = qsplit.tile([P, 2, QW], FP8, tag="qlo")
                nc.vector.tensor_sub(qlo[:, :, 0:w], qsum[:, :, 0:w],
                                     qhi[:, :, 0:w])
                return qhi, qlo

            ksum_tiles = {0: ksum_make(0, nc.vector, split_co=True)}
            kblks = {}

            def emit_epilogue(q0, w, po0, po1, pr, final=False):
                inv = small.tile([1, QW], F32R, tag="inv")
                with nc.allow_low_precision(
                    reason="TF32 rounding of softmax reciprocal"
                ):
                    nc.vector.reciprocal(inv[:, 0:w], pr[0:1, 0:w])
                pb = ps_b.tile([P, QW], F32, tag="b")
                nc.tensor.matmul(pb[:, 0:w], ones_r, inv[:, 0:w],
                                 start=True, stop=True)
                oo = outp.tile([P, 2, QW], F32, tag="oo")
                bs = small.tile([P, QW], F32, tag="bs")
                if final:
                    # tail latency: pipeline scale + writeback in halves
                    hw_ = w // 2
                    for h in range(2):
                        hs = slice(h * hw_, (h + 1) * hw_)
                        nc.vector.tensor_copy(bs[:, hs], pb[:, hs])
                        nc.vector.tensor_mul(oo[:, 0, hs], po0[:, hs],
                                             bs[:, hs])
                        nc.vector.tensor_mul(oo[:, 1, hs], po1[:, hs],
                                             bs[:, hs])
                        nc.sync.dma_start(
                            o3[:, :, q0 + h * hw_:q0 + (h + 1) * hw_],
                            oo[:, :, hs])
                else:
                    nc.vector.tensor_copy(bs[:, 0:w], pb[:, 0:w])
                    nc.vector.tensor_mul(oo[:, 0, 0:w], po0[:, 0:w],
                                         bs[:, 0:w])
                    nc.vector.tensor_mul(oo[:, 1, 0:w], po1[:, 0:w],
                                         bs[:, 0:w])
                    nc.sync.dma_start(o3[:, :, q0:q0 + w], oo[:, :, 0:w])

            pending = None
            # block 0 runs S in bf16 straight from the sums (no fp8 split):
            # its prep is one DVE add per K block, so the warmup is bounded
            # by DMA arrival, not by split chains
            qsum0 = sums.tile([P, 2, QW], BF16, tag="qsum0")
            for co in range(2):
                nc.vector.tensor_add(qsum0[:, co, :], qtile[:, co, 0:QW],
                                     qptile[:, co, 0:QW])
            qprepped = {}
            widths = [QW] * NQB
            starts = [sum(widths[:i]) for i in range(len(widths))]
            nblk = len(widths)
            for j in range(nblk):
                w = widths[j]
                q0 = starts[j]
                qhi, qlo = qprepped.pop(j) if j > 0 else (None, None)

                po0 = ps_o.tile([P, QW], F32, tag="o0")
                po1 = ps_o.tile([P, QW], F32, tag="o1")
                acc = accp.tile([P, QW], FP16, tag="acc")

                a_q = {}

                for ko in range(NKO):
                    jb, koff = divmod(ko, KPB)
                    if j == 0:
                        jt = ko // 2 + 1
                        if ko % 2 == 0 and jt < NQB:
                            # pos-add for K block jt (bf16, cheap, one op)
                            eng = nc.vector if jt < 4 else nc.gpsimd
                            ksum_tiles[jt] = ksum_make(jt, eng)
                        if ko >= 16 and ko % 2 == 0:
                            # fp8 hi/lo splits, due from block 1 on; Act
                            # and Pool do them so DVE keeps up with acc
                            js = (ko - 16) // 2
                            kblks[js] = ksplit_make(ksum_tiles[js])
                    if ko == 6 and j + 1 < nblk:
                        qprepped[j + 1] = q_prep(starts[j + 1],
                                                 widths[j + 1])

                    pss = ps_s.tile([P, QW], F32, tag="s")
                    ksl = slice(koff * P, (koff + 1) * P)
                    if j == 0:
                        ksum = ksum_tiles[jb]
                        for co in range(2):
                            nc.tensor.matmul(pss[:, 0:w], ksum[:, co, ksl],
                                             qsum0[:, co, 0:w],
                                             start=(co == 0), stop=(co == 1))
                    else:
                        khi, klo = kblks[jb]
                        nc.tensor.matmul(pss[:, 0:w], khi[:, :, ksl],
                                         qhi[:, :, 0:w],
                                         start=True, stop=False, perf_mode=DR)
                        nc.tensor.matmul(pss[:, 0:w], khi[:, :, ksl],
                                         qlo[:, :, 0:w],
                                         start=False, stop=False,
                                         perf_mode=DR)
                        nc.tensor.matmul(pss[:, 0:w], klo[:, :, ksl],
                                         qhi[:, :, 0:w],
                                         start=False, stop=True, perf_mode=DR)
                    a = atp.tile([P, QW], BF16, tag="a")
                    # exp is biased by a constant (cancels in the softmax
                    # division): logits reach ~14, and an unshifted exp
                    # overflows the fp16 row-sum accumulator
                    nc.scalar.activation(a[:, 0:w], pss[:, 0:w], AF.Exp,
                                         scale=SCALE, bias=exp_bias)
                    a_q[ko] = a
                    # fp16 row-sum accumulator on DVE (2x mode: all 2-byte)
                    if ko == 0:
                        nc.vector.tensor_copy(acc[:, 0:w], a[:, 0:w])
                    else:
                        nc.vector.tensor_add(acc[:, 0:w], acc[:, 0:w],
                                             a[:, 0:w])

                    if ko >= lag:
                        pko = ko - lag
                        av = a_q[pko]
                        nc.tensor.matmul(po0[:, 0:w], vtile[:, pko, 0:P],
                                         av[:, 0:w],
                                         start=(pko == 0), stop=False)
                        nc.tensor.matmul(po1[:, 0:w], vtile[:, pko, P:C],
                                         av[:, 0:w],
                                         start=(pko == 0),
                                         stop=(pko == NKO - 1))
                        del a_q[pko]

                    if ko == 1 and pending is not None:
                        # fold the previous block's partition accumulator
                        # here: its DVE chain has settled, so the PE never
                        # stalls on it at the block boundary
                        pq0, pw, ppo0, ppo1, pacc = pending
                        pr = ps_b.tile([P, QW], F32, tag="b")
                        nc.tensor.matmul(pr[0:2, 0:pw], ones_h,
                                         pacc[:, 0:pw],
                                         start=True, stop=True)
                        pending = (pq0, pw, ppo0, ppo1, pr)
                    if ko == 2 and pending is not None:
                        emit_epilogue(*pending)
                        pending = None

                # drain remaining lagged chunks; last closes the groups
                final = j == nblk - 1
                for pko in range(NKO - lag, NKO):
                    av = a_q[pko]
                    last = pko == NKO - 1
                    if final and last:
                        # the final denominator fold goes ahead of the last
                        # O pair: acc is ready, so the epilogue chain
                        # (recip -> pb -> muls -> dma) starts sooner
                        pr = ps_b.tile([P, QW], F32, tag="b")
                        nc.tensor.matmul(pr[0:2, 0:w], ones_h, acc[:, 0:w],
                                         start=True, stop=True)
                    nc.tensor.matmul(po0[:, 0:w], vtile[:, pko, 0:P],
                                     av[:, 0:w], start=False, stop=last)
                    nc.tensor.matmul(po1[:, 0:w], vtile[:, pko, P:C],
                                     av[:, 0:w], start=False, stop=last)
                    del a_q[pko]

                pending = (q0, w, po0, po1, acc)

            pq0, pw, ppo0, ppo1, pacc = pending
            emit_epilogue(pq0, pw, ppo0, ppo1, pr, final=True)

    nc.compile()
    return nc


def _get_nc():
    global _NC_CACHE
    if _NC_CACHE is None:
        _NC_CACHE = build_nc()
    return _NC_CACHE


def make_in_maps(queries, keys, values, q_pos_embedding, k_pos_embedding):
    bf16 = ml_dtypes.bfloat16
    queries = np.asarray(queries, dtype=np.float32)
    keys = np.asarray(keys, dtype=np.float32)
    values = np.asarray(values, dtype=np.float32)
    fp8 = ml_dtypes.float8_e4m3
    qpT = np.ascontiguousarray(
        np.asarray(q_pos_embedding, dtype=np.float32).reshape(N, C).T
    ).astype(fp8)
    kpT = np.ascontiguousarray(
        np.asarray(k_pos_embedding, dtype=np.float32).reshape(N, C).T
    ).astype(fp8)
    in_maps = []
    for b in range(B):
        vT = np.ascontiguousarray(values[b].reshape(C, N).T).astype(bf16)
        in_maps.append({
            "qt": np.ascontiguousarray(queries[b].reshape(C, N)).astype(bf16),
            "kt": np.ascontiguousarray(keys[b].reshape(C, N)).astype(bf16),
            "v": vT,
            "qp": qpT,
            "kp": kpT,
        })
    return in_maps


def kernel(queries, keys, values, q_pos_embedding, k_pos_embedding):
    nc = _get_nc()
    in_maps = make_in_maps(queries, keys, values, q_pos_embedding,
                           k_pos_embedding)
    res = run_bass_kernel_spmd(nc, in_maps, core_ids=list(range(B)))
    out = np.stack([r["o"].reshape(C, 64, 64) for r in res.results])
    return out.astype(np.float32)


def build_nc_trivial():
    """Same I/O signature, minimal work: used by test.py to subtract the
    per-call transfer/dispatch overhead from wall-clock timing."""
    nc = bacc.Bacc(None, target_bir_lowering=False)
    qt = nc.dram_tensor("qt", [C, N], BF16, kind="ExternalInput")
    kt = nc.dram_tensor("kt", [C, N], BF16, kind="ExternalInput")
    v = nc.dram_tensor("v", [N, C], BF16, kind="ExternalInput")
    qp = nc.dram_tensor("qp", [C, N], BF16, kind="ExternalInput")
    kp = nc.dram_tensor("kp", [C, N], BF16, kind="ExternalInput")
    o = nc.dram_tensor("o", [C, N], F32, kind="ExternalOutput")
    with tile.TileContext(nc) as tc:
        with tc.tile_pool(name="sb", bufs=2) as sb:
            t = sb.tile([P, 2, N], BF16, tag="t")
            nc.sync.dma_start(t, qt.rearrange("(co p) n -> p co n", p=P))
            nc.sync.dma_start(o.rearrange("(co p) n -> p co n", p=P), t)
    nc.compile()
    return nc
